# revision 2
# baseline (speedup 1.0000x reference)
"""Trainium2 Bass kernel for CustomHyperSemanticMessagePassing.

Hypergraph multi-head attention message passing, N=4096 nodes, E=4096 edges,
DEG=CARD=8, D=256, H=8 heads. Sharding: data-parallel over nodes (512/core).

Host: derives edge_of_node / node_of_edge index lists from the binary
incidence matrix, pre-combines the small projection weights, and pre-shards
the pair tensors. Device (per core): builds K/V/EK tables with PE matmuls
(replicated), then per 128-node tile gathers per-pair K|V rows with indirect
DMA and runs the attention (scores, exp, weighted sum, out-proj, relu).

Key identities used:
  k_pair = Wh[u] @ Wk.T + We[e] @ Wk.T + bk  -> gather(K_tab)[u] + gather(EK_tab)[e]
  v_pair = Wh[u] @ Wv.T + bv                 -> gather(V_tab)[u]
  softmax without max-subtraction (scores are O(1) bounded), so per-round
  partial exp sums / weighted sums combine by plain addition.
"""
import numpy as np

import bass_rust
import orjson
import concourse.bass as bass
import concourse.tile as tile
import concourse.bass_utils as bass_utils
import concourse.bass2jax as bass2jax
from concourse import mybir
from concourse.masks import make_identity

F32 = mybir.dt.float32
I32 = mybir.dt.int32

N, E, D, EDGE_DIM = 4096, 4096, 256, 64
H, DH, DEG, CARD = 8, 32, 8, 8
L = DEG * CARD
NCORES = 8
NSH = N // NCORES          # nodes per core
NT = NSH // 128            # 128-node tiles per core


# ---------------------------------------------------------------------------
# walrus workaround: this build accepts only one sync-wait per instruction;
# split extras into injected single-wait NoOps at the BIR-JSON level.
_ORIG_COMPILE = bass_utils.compile_bir_kernel
_ctr = [0]


def _split_multiwaits(bir_json: bytes) -> bytes:
    bir = orjson.loads(bir_json)
    changed = False
    for f in bir.get("functions", []):
        for blk in f.get("blocks", []):
            out = []
            for ins in blk.get("instructions", []):
                si = ins.get("sync_info")
                waits = (si or {}).get("on_wait") or []
                if len(waits) > 1 and ins.get("engine") not in (None, "Unassigned"):
                    changed = True
                    for w in waits[:-1]:
                        _ctr[0] += 1
                        out.append({
                            "debug": ins.get("debug"),
                            "engine": ins["engine"],
                            "ins": [], "outs": [],
                            "name": f"WSPLIT-{_ctr[0]}",
                            "opcode": "NoOp",
                            "sync_info": {"on_wait": [w], "on_update": []},
                        })
                    si["on_wait"] = waits[-1:]
                out.append(ins)
            blk["instructions"] = out
    return orjson.dumps(bir) if changed else bir_json


def _patched_compile(bir_json, tmpdir, neff_name="file.neff"):
    return _ORIG_COMPILE(_split_multiwaits(bytes(bir_json)), tmpdir,
                         neff_name=neff_name)


def _install_patch():
    bass_utils.compile_bir_kernel = _patched_compile
    bass2jax.compile_bir_kernel = _patched_compile


_install_patch()


# ---------------------------------------------------------------------------
def build_nc():
    nc = bass.Bass(num_devices=NCORES)
    # replicated inputs
    xT = nc.declare_dram_parameter("xT", [D, N], F32, isOutput=False)
    eaT = nc.declare_dram_parameter("eaT", [EDGE_DIM, E], F32, isOutput=False)
    wkc = nc.declare_dram_parameter("wkc", [D, D], F32, isOutput=False)
    wvc = nc.declare_dram_parameter("wvc", [D, D], F32, isOutput=False)
    wqc = nc.declare_dram_parameter("wqc", [D, D], F32, isOutput=False)
    wek = nc.declare_dram_parameter("wek", [EDGE_DIM, D], F32, isOutput=False)
    owT = nc.declare_dram_parameter("owT", [D, D], F32, isOutput=False)
    bkv_b = nc.declare_dram_parameter("bkv_b", [128, 2 * D], F32, isOutput=False)
    bq_b = nc.declare_dram_parameter("bq_b", [128, D], F32, isOutput=False)
    bk_b = nc.declare_dram_parameter("bk_b", [128, D], F32, isOutput=False)
    bo_b = nc.declare_dram_parameter("bo_b", [128, D], F32, isOutput=False)
    # per-core inputs
    xT_own = nc.declare_dram_parameter("xT_own", [D, NSH], F32, isOutput=False)
    pu = nc.declare_dram_parameter("pu", [NSH, L], I32, isOutput=False)
    pe = nc.declare_dram_parameter("pe", [NSH, DEG], I32, isOutput=False)
    # output
    out = nc.declare_dram_parameter("out", [NSH, D], F32, isOutput=True)
    # internal tables
    kv_tab = nc.dram_tensor("kv_tab", [N, 2 * D], F32)
    ek_tab = nc.dram_tensor("ek_tab", [E, D], F32)

    with tile.TileContext(nc) as tc, \
         tc.tile_pool(name="wpool", bufs=1) as wp, \
         tc.tile_pool(name="xpool", bufs=3) as xp, \
         tc.tile_pool(name="tpool", bufs=3) as tp, \
         tc.tile_pool(name="qpool", bufs=NT) as qp, \
         tc.tile_pool(name="gpool", bufs=3) as gp, \
         tc.tile_pool(name="apool", bufs=2) as ap_, \
         tc.tile_pool(name="cpool", bufs=2) as cp, \
         tc.tile_pool(name="psA", bufs=2, space="PSUM") as psA, \
         tc.tile_pool(name="psB", bufs=2, space="PSUM") as psB, \
         tc.tile_pool(name="psC", bufs=2, space="PSUM") as psC:

        # ---- load weights/biases (resident) ----
        wk_t = wp.tile([128, 2, D], F32)
        nc.sync.dma_start(out=wk_t[:], in_=wkc[:].rearrange("(c k) o -> k c o", c=2))
        wv_t = wp.tile([128, 2, D], F32)
        nc.sync.dma_start(out=wv_t[:], in_=wvc[:].rearrange("(c k) o -> k c o", c=2))
        wq_t = wp.tile([128, 2, D], F32)
        nc.sync.dma_start(out=wq_t[:], in_=wqc[:].rearrange("(c k) o -> k c o", c=2))
        wek_t = wp.tile([EDGE_DIM, D], F32)
        nc.sync.dma_start(out=wek_t[:], in_=wek[:])
        owT_t = wp.tile([128, 2, D], F32)
        nc.sync.dma_start(out=owT_t[:], in_=owT[:].rearrange("(c k) o -> k c o", c=2))
        bkv_t = wp.tile([128, 2 * D], F32)
        nc.sync.dma_start(out=bkv_t[:], in_=bkv_b[:])
        bq_t = wp.tile([128, D], F32)
        nc.sync.dma_start(out=bq_t[:], in_=bq_b[:])
        bk_t = wp.tile([128, D], F32)
        nc.sync.dma_start(out=bk_t[:], in_=bk_b[:])
        bo_t = wp.tile([128, D], F32)
        nc.sync.dma_start(out=bo_t[:], in_=bo_b[:])
        ident = wp.tile([128, 128], F32)
        make_identity(nc, ident[:])

        # ---- phase T: build KV table ----
        for m in range(N // 128):
            xt = xp.tile([128, 2, 128], F32, tag="xt")
            nc.sync.dma_start(
                out=xt[:],
                in_=xT[:, bass.ts(m, 128)].rearrange("(c k) n -> k c n", c=2))
            pkv = psA.tile([128, 2 * D], F32, space="PSUM", tag="pkv")
            nc.tensor.matmul(out=pkv[:, 0:D], lhsT=xt[:, 0, :], rhs=wk_t[:, 0, :],
                             start=True, stop=False)
            nc.tensor.matmul(out=pkv[:, 0:D], lhsT=xt[:, 1, :], rhs=wk_t[:, 1, :],
                             start=False, stop=True)
            nc.tensor.matmul(out=pkv[:, D:2 * D], lhsT=xt[:, 0, :], rhs=wv_t[:, 0, :],
                             start=True, stop=False)
            nc.tensor.matmul(out=pkv[:, D:2 * D], lhsT=xt[:, 1, :], rhs=wv_t[:, 1, :],
                             start=False, stop=True)
            kv_sb = tp.tile([128, 2 * D], F32, tag="kvsb")
            nc.vector.tensor_tensor(out=kv_sb[:], in0=pkv[:], in1=bkv_t[:],
                                    op=mybir.AluOpType.add)
            nc.sync.dma_start(out=kv_tab[bass.ts(m, 128), :], in_=kv_sb[:])

        # ---- phase T: build EK table ----
        for m in range(E // 128):
            et = xp.tile([EDGE_DIM, 128], F32, tag="et")
            nc.sync.dma_start(out=et[:], in_=eaT[:, bass.ts(m, 128)])
            pek = psB.tile([128, D], F32, space="PSUM", tag="p256")
            nc.tensor.matmul(out=pek[:], lhsT=et[:], rhs=wek_t[:],
                             start=True, stop=True)
            ek_sb = tp.tile([128, D], F32, tag="eksb")
            nc.vector.tensor_tensor(out=ek_sb[:], in0=pek[:], in1=bk_t[:],
                                    op=mybir.AluOpType.add)
            nc.sync.dma_start(out=ek_tab[bass.ts(m, 128), :], in_=ek_sb[:])

        # ---- phase T: q for own nodes (kept in SBUF) ----
        q_tiles = []
        for t in range(NT):
            xq = xp.tile([128, 2, 128], F32, tag="xq")
            nc.sync.dma_start(
                out=xq[:],
                in_=xT_own[:, bass.ts(t, 128)].rearrange("(c k) n -> k c n", c=2))
            pq = psB.tile([128, D], F32, space="PSUM", tag="p256")
            nc.tensor.matmul(out=pq[:], lhsT=xq[:, 0, :], rhs=wq_t[:, 0, :],
                             start=True, stop=False)
            nc.tensor.matmul(out=pq[:], lhsT=xq[:, 1, :], rhs=wq_t[:, 1, :],
                             start=False, stop=True)
            q_t = qp.tile([128, D], F32, tag=f"q{t}")
            nc.vector.tensor_tensor(out=q_t[:], in0=pq[:], in1=bq_t[:],
                                    op=mybir.AluOpType.add)
            q_tiles.append(q_t)

        # ---- phase A: attention per 128-node tile ----
        for t in range(NT):
            q_t = q_tiles[t]
            pu_t = ap_.tile([128, L], I32, tag="put")
            nc.sync.dma_start(out=pu_t[:], in_=pu[bass.ts(t, 128), :])
            pe_t = ap_.tile([128, DEG], I32, tag="pet")
            nc.sync.dma_start(out=pe_t[:], in_=pe[bass.ts(t, 128), :])

            ctx_r = cp.tile([128, DEG, D], F32, tag="ctxr")
            z_r = cp.tile([128, DEG, H], F32, tag="zr")

            for d in range(DEG):
                kvr = gp.tile([128, CARD, 2 * D], F32, tag="kvr")
                for c in range(CARD):
                    nc.gpsimd.indirect_dma_start(
                        out=kvr[:, c, :], out_offset=None, in_=kv_tab[:],
                        in_offset=bass.IndirectOffsetOnAxis(
                            ap=pu_t[:, d * CARD + c:d * CARD + c + 1], axis=0))
                ek_g = gp.tile([128, D], F32, tag="ekg")
                nc.gpsimd.indirect_dma_start(
                    out=ek_g[:], out_offset=None, in_=ek_tab[:],
                    in_offset=bass.IndirectOffsetOnAxis(
                        ap=pe_t[:, d:d + 1], axis=0))

                # qek[p,h] = sum_d q[p,h,:] * ek[p,h,:]
                prode = ap_.tile([128, D], F32, tag="prode")
                nc.vector.tensor_tensor(out=prode[:], in0=ek_g[:], in1=q_t[:],
                                        op=mybir.AluOpType.mult)
                qek = ap_.tile([128, H], F32, tag="qek")
                nc.vector.tensor_reduce(
                    out=qek[:], in_=prode[:].rearrange("p (h e) -> p h e", h=H),
                    axis=mybir.AxisListType.X, op=mybir.AluOpType.add)

                # s[p,c,h] = sum_e q[p,h,e] * K[p,c,h,e]  (+ qek)
                prodk = ap_.tile([128, CARD, D], F32, tag="prodk")
                nc.vector.tensor_tensor(
                    out=prodk[:], in0=kvr[:, :, 0:D],
                    in1=q_t[:].unsqueeze(1).to_broadcast([128, CARD, D]),
                    op=mybir.AluOpType.mult)
                s_d = ap_.tile([128, CARD, H], F32, tag="sd")
                nc.vector.tensor_reduce(
                    out=s_d[:],
                    in_=prodk[:].rearrange("p c (h e) -> p c h e", h=H),
                    axis=mybir.AxisListType.X, op=mybir.AluOpType.add)
                nc.vector.tensor_tensor(
                    out=s_d[:], in0=s_d[:],
                    in1=qek[:].unsqueeze(1).to_broadcast([128, CARD, H]),
                    op=mybir.AluOpType.add)

                # w = exp(s), z[p,h] = sum_c w[p,c,h]
                w_d = ap_.tile([128, CARD, H], F32, tag="wd")
                nc.scalar.activation(out=w_d[:], in_=s_d[:],
                                     func=mybir.ActivationFunctionType.Exp)
                nc.vector.tensor_reduce(
                    out=z_r[:, d, :], in_=w_d[:].transpose([0, 2, 1]),
                    axis=mybir.AxisListType.X, op=mybir.AluOpType.add)

                # ctx_r[p,d,:] = sum_c w[p,c,h] * V[p,c,h,e]
                wv = ap_.tile([128, CARD, D], F32, tag="wv")
                nc.vector.tensor_tensor(
                    out=wv[:].rearrange("p c (h e) -> p c h e", h=H),
                    in0=kvr[:, :, D:2 * D].rearrange("p c (h e) -> p c h e", h=H),
                    in1=w_d[:].unsqueeze(3).to_broadcast([128, CARD, H, DH]),
                    op=mybir.AluOpType.mult)
                nc.vector.tensor_reduce(
                    out=ctx_r[:, d, :], in_=wv[:].transpose([0, 2, 1]),
                    axis=mybir.AxisListType.X, op=mybir.AluOpType.add)

            # combine rounds
            ctx = tp.tile([128, D], F32, tag="ctx")
            nc.vector.tensor_reduce(
                out=ctx[:], in_=ctx_r[:].transpose([0, 2, 1]),
                axis=mybir.AxisListType.X, op=mybir.AluOpType.add)
            zsum = ap_.tile([128, H], F32, tag="zsum")
            nc.vector.tensor_reduce(
                out=zsum[:], in_=z_r[:].transpose([0, 2, 1]),
                axis=mybir.AxisListType.X, op=mybir.AluOpType.add)
            zrec = ap_.tile([128, H], F32, tag="zrec")
            nc.vector.reciprocal(out=zrec[:], in_=zsum[:])
            ctxn = tp.tile([128, D], F32, tag="ctxn")
            nc.vector.tensor_tensor(
                out=ctxn[:].rearrange("p (h e) -> p h e", h=H),
                in0=ctx[:].rearrange("p (h e) -> p h e", h=H),
                in1=zrec[:].unsqueeze(2).to_broadcast([128, H, DH]),
                op=mybir.AluOpType.mult)

            # out-proj: transpose ctxn, then PE matmul, bias, relu
            ctxT = tp.tile([128, 2, 128], F32, tag="ctxT")
            for ch in range(2):
                ptr = psC.tile([128, 128], F32, space="PSUM", tag="ptr")
                nc.tensor.transpose(out=ptr[:], in_=ctxn[:, bass.ts(ch, 128)],
                                    identity=ident[:])
                nc.scalar.copy(out=ctxT[:, ch, :], in_=ptr[:])
            po = psB.tile([128, D], F32, space="PSUM", tag="p256")
            nc.tensor.matmul(out=po[:], lhsT=ctxT[:, 0, :], rhs=owT_t[:, 0, :],
                             start=True, stop=False)
            nc.tensor.matmul(out=po[:], lhsT=ctxT[:, 1, :], rhs=owT_t[:, 1, :],
                             start=False, stop=True)
            ob = tp.tile([128, D], F32, tag="ob")
            nc.vector.tensor_tensor(out=ob[:], in0=po[:], in1=bo_t[:],
                                    op=mybir.AluOpType.add)
            o_sb = tp.tile([128, D], F32, tag="osb")
            nc.scalar.activation(out=o_sb[:], in_=ob[:],
                                 func=mybir.ActivationFunctionType.Relu)
            nc.sync.dma_start(out=out[bass.ts(t, 128), :], in_=o_sb[:])

    return nc


# ---------------------------------------------------------------------------
def host_prep(x, incidence, edge_attr, W_lin, W_edge,
              in_proj_w, in_proj_b, out_proj_w, out_proj_b):
    x = np.asarray(x, np.float32)
    inc = np.asarray(incidence, np.float32)
    ea = np.asarray(edge_attr, np.float32)
    W_lin = np.asarray(W_lin, np.float32)
    W_edge = np.asarray(W_edge, np.float32)
    in_proj_w = np.asarray(in_proj_w, np.float32)
    in_proj_b = np.asarray(in_proj_b, np.float32)
    out_proj_w = np.asarray(out_proj_w, np.float32)
    out_proj_b = np.asarray(out_proj_b, np.float32)

    # index lists from incidence (order within a node's pair set is irrelevant:
    # attention is permutation-invariant over the L pairs)
    eon = np.nonzero(inc.T)[1].reshape(N, DEG).astype(np.int32)   # edge_of_node
    noe = np.nonzero(inc)[1].reshape(E, CARD).astype(np.int32)    # node_of_edge
    pair_u = noe[eon].reshape(N, L).astype(np.int32)
    pair_e = eon

    Wq, Wk, Wv = in_proj_w[0:D], in_proj_w[D:2 * D], in_proj_w[2 * D:3 * D]
    bq, bk, bv = in_proj_b[0:D], in_proj_b[D:2 * D], in_proj_b[2 * D:3 * D]
    scale = 1.0 / np.sqrt(np.float32(DH))

    wkc = (W_lin @ Wk.T).astype(np.float32)
    wvc = (W_lin @ Wv.T).astype(np.float32)
    wqc = (W_lin @ Wq.T * scale).astype(np.float32)
    wek = (W_edge @ Wk.T).astype(np.float32)
    owT = out_proj_w.T.copy().astype(np.float32)

    rep = dict(
        xT=np.ascontiguousarray(x.T),
        eaT=np.ascontiguousarray(ea.T),
        wkc=wkc, wvc=wvc, wqc=wqc, wek=wek, owT=owT,
        bkv_b=np.broadcast_to(np.concatenate([np.zeros(D, np.float32), bv]),
                              (128, 2 * D)).copy(),
        bq_b=np.broadcast_to(bq * scale, (128, D)).copy(),
        bk_b=np.broadcast_to(bk, (128, D)).copy(),
        bo_b=np.broadcast_to(out_proj_b, (128, D)).copy(),
    )
    per_core = []
    for c in range(NCORES):
        sl = slice(c * NSH, (c + 1) * NSH)
        m = dict(rep)
        m["xT_own"] = np.ascontiguousarray(x.T[:, sl])
        m["pu"] = pair_u[sl]
        m["pe"] = pair_e[sl]
        per_core.append(m)
    return per_core


_CACHE = {}


def kernel(x, incidence, edge_attr, W_lin, W_edge,
           in_proj_w, in_proj_b, out_proj_w, out_proj_b, deg, card):
    assert int(deg) == DEG and int(card) == CARD
    in_maps = host_prep(x, incidence, edge_attr, W_lin, W_edge,
                        in_proj_w, in_proj_b, out_proj_w, out_proj_b)
    if "nc" not in _CACHE:
        _CACHE["nc"] = build_nc()
    from concourse.bass_utils import run_bass_kernel_spmd
    res = run_bass_kernel_spmd(_CACHE["nc"], in_maps, list(range(NCORES)))
    return np.concatenate([res.results[c]["out"] for c in range(NCORES)], axis=0)
